# revision 1
# baseline (speedup 1.0000x reference)
"""Trainium2 Bass kernel for DUPN-style LSTM + windowed-softmax attention pooling.

Math (per batch element b):
  LSTM over T=128 steps (torch gate order), hidden H=512, input D=256.
  a[t] = sigmoid(x[t]·u1 + h[t]·u2), u1 = (v1@A1)^T, u2 = (v1@A2)^T  (folded)
  out[b,k,:] = softmax-pooled sum of h[t] over window t <= t_k, for 4 slots.

Sharding: data-parallel over batch, 32 per core x 8 cores, weights replicated.

Per-core device schedule:
  - xw = x@W_ih^T + bias precomputed in row-chunks of 128 rows (4 timesteps),
    fused into the loop as a prefetch, kept in an SBUF ring. Bias folded in
    via a K=1 ones-row matmul. Computed in two [128, 1024] halves to fit PSUM.
  - Per step: z [32, 2048] accumulated in a 4-bank PSUM tile: 4 identity
    matmuls inject xw rows (K=32), then 16 k-pass matmuls add h @ W_hh^T.
    All matmul outputs start at PSUM partition 0 (walrus emits col_grp=0xf
    only; non-zero dst partitions are unencodable).
  - Matmul operands are fp32r (fp32 rounded to 11 mantissa bits; full PE rate
    at N>=256). walrus requires producer dtype = fp32r, so matmul-feeding
    tiles are declared fp32r and written by converting copies.
  - Gates are free-dim slices of z (order i,f,o,g): sigmoid on [:, 0:1536],
    tanh on [:, 1536:2048]; c/h updates on DVE, everything at partition 0.
  - h transposed to hT via 4 PE transposes into hsT_store (fp32r), which is
    the next step's matmul stationary and the pooling source.
  - Post-loop: a = sigmoid(a1+a2), windowed softmax with host-built masks,
    pooling via per-b [4,T]@[T,H] matmuls.
"""
import sys

if "/opt/trn_rl_repo" not in sys.path:
    sys.path.insert(0, "/opt/trn_rl_repo")

import numpy as np
import concourse.bass as bass
import concourse.bacc as bacc
import concourse.tile as tile
from concourse import mybir
from concourse.bass_utils import run_bass_kernel_spmd
from contextlib import ExitStack

F32 = mybir.dt.float32
F32R = mybir.dt.float32r
AFT = mybir.ActivationFunctionType
ALU = mybir.AluOpType

T, BF, D, H, K, NC = 128, 256, 256, 512, 4, 8
BL = BF // NC          # 32 batch per core
G = 4 * H              # 2048
NEG_INF = -1e9

_cached = {}


def _build_program(t_steps=T):
    nc = bacc.Bacc()
    # ---- DRAM I/O (fp32r where feeding matmuls; same bytes as fp32) ----
    d_xT = nc.declare_dram_parameter("xT", [D, t_steps * BL], F32R, isOutput=False)
    d_wih = nc.declare_dram_parameter("wih", [D, G], F32R, isOutput=False)
    d_whh = nc.declare_dram_parameter("whh", [H, G], F32R, isOutput=False)
    d_biasrow = nc.declare_dram_parameter("biasrow", [1, G], F32R, isOutput=False)
    d_ones = nc.declare_dram_parameter("onesrow", [1, 128], F32R, isOutput=False)
    d_u1t = nc.declare_dram_parameter("u1t", [128, 2 * (D // 128)], F32R, isOutput=False)
    d_u2b = nc.declare_dram_parameter("u2b", [BL, H], F32, isOutput=False)
    d_i32s = nc.declare_dram_parameter("i32s", [128, 32], F32, isOutput=False)
    d_i128 = nc.declare_dram_parameter("i128", [128, 128], F32, isOutput=False)
    d_maskneg = nc.declare_dram_parameter("maskneg", [BL, K * t_steps], F32, isOutput=False)
    d_valid = nc.declare_dram_parameter("valid", [BL, K], F32, isOutput=False)
    d_out = nc.declare_dram_parameter("out", [BL * K, H], F32, isOutput=True)

    NRC = t_steps // 4     # row chunks of 128 rows (4 timesteps each)

    with tile.TileContext(nc) as tc, ExitStack() as ctx:
        nv, ns, nt, ng = nc.vector, nc.scalar, nc.tensor, nc.gpsimd

        consts = ctx.enter_context(tc.tile_pool(name="consts", bufs=1))
        big = ctx.enter_context(tc.tile_pool(name="big", bufs=1))

        # ---- load constants ----
        wih_sb = [consts.tile([128, G], F32R, tag=f"wih{i}", name=f"wih{i}")
                  for i in range(2)]
        for i in range(2):
            nc.sync.dma_start(wih_sb[i][:], d_wih[128 * i:128 * (i + 1), :])
        whh_sb = [consts.tile([128, G], F32R, tag=f"whh{i}", name=f"whh{i}")
                  for i in range(4)]
        for i in range(4):
            nc.sync.dma_start(whh_sb[i][:], d_whh[128 * i:128 * (i + 1), :])
        biasrow_sb = consts.tile([1, G], F32R, tag="biasrow")
        nc.sync.dma_start(biasrow_sb[:], d_biasrow[:])
        ones_sb = consts.tile([1, 128], F32R, tag="ones")
        nc.sync.dma_start(ones_sb[:], d_ones[:])
        u1t_sb = consts.tile([128, 4], F32R, tag="u1t")
        nc.sync.dma_start(u1t_sb[:], d_u1t[:])
        u2b_sb = consts.tile([BL, H], F32, tag="u2b")
        nc.sync.dma_start(u2b_sb[:], d_u2b[:])
        i32s_r = consts.tile([128, 32], F32R, tag="i32s_r")
        nc.sync.dma_start(i32s_r[:], d_i32s[:].bitcast(F32R))
        i32s_f = consts.tile([128, 32], F32, tag="i32s_f")
        nc.sync.dma_start(i32s_f[:], d_i32s[:])
        i128_r = consts.tile([128, 128], F32R, tag="i128_r")
        nc.sync.dma_start(i128_r[:], d_i128[:].bitcast(F32R))
        maskneg_sb = consts.tile([BL, K * t_steps], F32, tag="maskneg")
        nc.sync.dma_start(maskneg_sb[:], d_maskneg[:])
        valid_sb = consts.tile([BL, K], F32, tag="valid")
        nc.sync.dma_start(valid_sb[:], d_valid[:])

        # ---- persistent state ----
        hsT = big.tile([128, t_steps * 128], F32R, tag="hsT")      # [p, t*128+c*32+b]
        c_sb = big.tile([BL, H], F32, tag="c")
        a1ch = big.tile([128, NRC], F32, tag="a1ch")               # a1 by row-chunk
        a2_sb = big.tile([BL, t_steps], F32, tag="a2")

        # ---- loop pools ----
        loop_ctx = ExitStack()
        xt_pool = loop_ctx.enter_context(tc.tile_pool(name="xt", bufs=2))
        xw_pool = loop_ctx.enter_context(tc.tile_pool(name="xw", bufs=2))
        gate_pool = loop_ctx.enter_context(tc.tile_pool(name="gate", bufs=2))
        tmp_pool = loop_ctx.enter_context(tc.tile_pool(name="tmp", bufs=2))
        h_pool = loop_ctx.enter_context(tc.tile_pool(name="h", bufs=2))
        scr_pool = loop_ctx.enter_context(tc.tile_pool(name="scr", bufs=1))
        ps_xw = loop_ctx.enter_context(tc.tile_pool(name="ps_xw", bufs=1, space="PSUM"))
        ps_z = loop_ctx.enter_context(tc.tile_pool(name="ps_z", bufs=1, space="PSUM"))
        ps_hT = loop_ctx.enter_context(tc.tile_pool(name="ps_hT", bufs=1, space="PSUM"))
        ps_a1 = loop_ctx.enter_context(tc.tile_pool(name="ps_a1", bufs=1, space="PSUM"))

        def emit_xw_chunk(r):
            """xw rows 128r..128r+128 (timesteps 4r..4r+3) -> xw ring + a1 col r.

            Two [128, 1024] PSUM halves (2 banks each, bufs=2) to stay in
            budget: ps_xw 2x2 + ps_z 4 + ps_hT 1 + ps_a1 1 = 8 banks.
            """
            xtc = [xt_pool.tile([128, 128], F32R, tag=f"xtc{kd}", name=f"xtc{kd}_{r}")
                   for kd in range(2)]
            for kd in range(2):
                nc.sync.dma_start(xtc[kd][:],
                                  d_xT[128 * kd:128 * (kd + 1), 128 * r:128 * (r + 1)])
            xw = xw_pool.tile([128, G], F32R, tag="xw")
            for half in range(2):
                pxw = ps_xw.tile([128, 1024], F32, tag="pxw")
                for kd in range(2):
                    for n in range(2):
                        nn_ = 2 * half + n
                        nt.matmul(pxw[:, 512 * n:512 * (n + 1)], xtc[kd],
                                  wih_sb[kd][:, 512 * nn_:512 * (nn_ + 1)],
                                  start=(kd == 0), stop=False)
                for n in range(2):
                    nn_ = 2 * half + n
                    nt.matmul(pxw[:, 512 * n:512 * (n + 1)], ones_sb[:],
                              biasrow_sb[:, 512 * nn_:512 * (nn_ + 1)],
                              start=False, stop=True)
                if half == 0:
                    ns.copy(xw[:, 0:1024], pxw[:])
                else:
                    nv.tensor_copy(xw[:, 1024:2048], pxw[:])
            pa1 = ps_a1.tile([128, 2], F32)
            for kd in range(2):
                nt.matmul(pa1[:], xtc[kd], u1t_sb[:, 2 * kd:2 * kd + 2],
                          start=(kd == 0), stop=(kd == 1))
            ns.copy(a1ch[:, r:r + 1], pa1[:, 0:1])
            return xw

        xw_tiles = {0: emit_xw_chunk(0)}

        for t in range(t_steps):
            r, t4 = divmod(t, 4)
            xw = xw_tiles[r]
            pz = ps_z.tile([BL, G], F32, tag="pz")
            # n-chunk outer: chunk n finishes early so gates can start sooner
            for n in range(4):
                nt.matmul(pz[:, 512 * n:512 * (n + 1)],
                          i32s_r[32 * t4:32 * (t4 + 1), :],
                          xw[32 * t4:32 * (t4 + 1), 512 * n:512 * (n + 1)],
                          start=True, stop=(t == 0),
                          tile_position=(32 * t4, 0))
                if t > 0:
                    for k in range(4):
                        nt.matmul(
                            pz[:, 512 * n:512 * (n + 1)],
                            hsT[:, (t - 1) * 128 + 32 * k:(t - 1) * 128 + 32 * (k + 1)],
                            whh_sb[k][:, 512 * n:512 * (n + 1)],
                            start=False, stop=(k == 3))
            # gates: z cols [i(0:512) f(512:1024) o(1024:1536) g(1536:2048)]
            sg = gate_pool.tile([BL, 1536], F32, tag="sg")
            ns.activation(sg[:], pz[:, 0:1536], AFT.Sigmoid)
            gg = gate_pool.tile([BL, 512], F32, tag="gg")
            ns.activation(gg[:], pz[:, 1536:2048], AFT.Tanh)
            tig = tmp_pool.tile([BL, H], F32, tag="tig")
            nv.tensor_tensor(tig[:], sg[:, 0:512], gg[:], op=ALU.mult)
            if t == 0:
                nv.tensor_copy(c_sb[:], tig[:])
            else:
                tfc = tmp_pool.tile([BL, H], F32, tag="tfc")
                nv.tensor_tensor(tfc[:], sg[:, 512:1024], c_sb[:], op=ALU.mult)
                nv.tensor_tensor(c_sb[:], tfc[:], tig[:], op=ALU.add)
            tcs = tmp_pool.tile([BL, H], F32, tag="tcs")
            ns.activation(tcs[:], c_sb[:], AFT.Tanh)
            h_t = h_pool.tile([BL, H], F32, tag="h")
            nv.tensor_tensor(h_t[:], sg[:, 1024:1536], tcs[:], op=ALU.mult)
            # a2[t] = h . u2  (per-partition dot)
            scr = scr_pool.tile([BL, H], F32, tag="scr")
            nv.scalar_tensor_tensor(scr[:], h_t[:], 1.0, u2b_sb[:],
                                    op0=ALU.bypass, op1=ALU.mult,
                                    accum_out=a2_sb[:, t:t + 1])
            # transpose h -> hsT[:, t*128:(t+1)*128] (converts to fp32r)
            phT = ps_hT.tile([128, 128], F32, tag="phT")
            for c in range(4):
                nt.transpose(phT[:, 32 * c:32 * (c + 1)],
                             h_t[:, 128 * c:128 * (c + 1)], i32s_f[0:32, :])
            ns.copy(hsT[:, t * 128:(t + 1) * 128], phT[:])
            # prefetch next xw chunk (3 steps of slack before it's consumed)
            if t4 == 0 and r + 1 < NRC:
                xw_tiles[r + 1] = emit_xw_chunk(r + 1)
                xw_tiles.pop(r - 1, None)

        loop_ctx.close()

        # ---- post-loop: attention scores + softmax + pooling ----
        post = ctx.enter_context(tc.tile_pool(name="post", bufs=1))
        ps_t = ctx.enter_context(tc.tile_pool(name="ps_t", bufs=2, space="PSUM"))
        ps_pool = ctx.enter_context(tc.tile_pool(name="ps_pool", bufs=4, space="PSUM"))
        stg_pool = ctx.enter_context(tc.tile_pool(name="stg", bufs=4))
        hsb_pool = ctx.enter_context(tc.tile_pool(name="hsb", bufs=2))

        # a1 assembly: a1bp[b, 4r+c] = a1ch[32c+b, r]
        a1bp = post.tile([BL, t_steps], F32, tag="a1bp")
        for c in range(4):
            nv.tensor_copy(a1bp[:].rearrange("b (r c) -> b r c", c=4)[:, :, c],
                           a1ch[32 * c:32 * (c + 1), :])
        abp = post.tile([BL, t_steps], F32, tag="abp")
        nv.tensor_tensor(abp[:], a1bp[:], a2_sb[:], op=ALU.add)
        ns.activation(abp[:], abp[:], AFT.Sigmoid)

        # softmax per slot k -> wT [t, 4b+k] (fp32r for the pooling matmul)
        wT = post.tile([t_steps, K * BL], F32R, tag="wT")
        for k in range(K):
            sc = post.tile([BL, t_steps], F32, tag=f"sc{k}")
            nv.tensor_tensor(sc[:], abp[:],
                             maskneg_sb[:, t_steps * k:t_steps * (k + 1)], op=ALU.add)
            mneg = post.tile([BL, 1], F32, tag=f"mneg{k}")
            nv.tensor_reduce(mneg[:], sc[:], axis=mybir.AxisListType.X,
                             op=ALU.max, negate=True)
            ek = post.tile([BL, t_steps], F32, tag=f"ek{k}")
            sk = post.tile([BL, 1], F32, tag=f"sk{k}")
            ns.activation(ek[:], sc[:], AFT.Exp, bias=mneg[:], accum_out=sk[:])
            rk = post.tile([BL, 1], F32, tag=f"rk{k}")
            nv.reciprocal(rk[:], sk[:])
            wk = post.tile([BL, t_steps], F32, tag=f"wk{k}")
            nv.tensor_scalar(out=wk[:], in0=ek[:], scalar1=rk[:],
                             scalar2=valid_sb[:, k:k + 1], op0=ALU.mult, op1=ALU.mult)
            # transpose into wT columns k::4  (wT[t, 4b+k])
            pwT = ps_t.tile([128, 32], F32, tag="pwT")
            nt.transpose(pwT[0:t_steps, :], wk[:], i32s_f[0:32, :])
            nv.tensor_copy(wT[:].rearrange("t (b k) -> t b k", k=4)[:, :, k],
                           pwT[0:t_steps, :])

        # pooling: per b, rebuild hs_b [t, h] via 4 PE transposes, then [4,T]@[T,H]
        hsT_r = hsT[:].rearrange("p (t c b) -> p t c b", c=4, b=BL)
        for b in range(BL):
            hsb = hsb_pool.tile([t_steps, H], F32R, tag="hsb")
            for c in range(4):
                pt = ps_t.tile([128, 128], F32R, tag="pt")
                nt.transpose(pt[0:t_steps, :], hsT_r[:, :, c, b], i128_r[:])
                if c % 2 == 0:
                    ns.copy(hsb[:, 128 * c:128 * (c + 1)], pt[0:t_steps, :])
                else:
                    nv.tensor_copy(hsb[:, 128 * c:128 * (c + 1)], pt[0:t_steps, :])
            pp = ps_pool.tile([K, H], F32, tag="pp")
            nt.matmul(pp[:], wT[0:t_steps, 4 * b:4 * (b + 1)], hsb[:],
                      start=True, stop=True)
            so = stg_pool.tile([K, H], F32, tag="so")
            ns.copy(so[:], pp[:])
            nc.sync.dma_start(d_out[K * b:K * (b + 1), :], so[:])

    nc.compile()
    return nc


def _host_prep(x, W_ih, W_hh, b_ih, b_hh, A1, A2, v1, lengths, label_len):
    assert int(label_len) == K
    perm = np.concatenate([np.arange(0, 512), np.arange(512, 1024),
                           np.arange(1536, 2048), np.arange(1024, 1536)])
    wih = np.ascontiguousarray(W_ih[perm].T, dtype=np.float32)          # [256, 2048]
    whh = np.ascontiguousarray(W_hh[perm].T, dtype=np.float32)          # [512, 2048]
    biasrow = ((b_ih + b_hh)[perm]).astype(np.float32).reshape(1, G)
    u1 = (v1 @ A1)[0].astype(np.float32)                                # [256]
    u2 = (v1 @ A2)[0].astype(np.float32)                                # [512]
    u1t = np.zeros((128, 4), dtype=np.float32)                          # [128, 4]
    u1t[:, 0] = u1[0:128]
    u1t[:, 2] = u1[128:256]
    u2b = np.ascontiguousarray(np.broadcast_to(u2, (BL, H)))            # [32, 512]
    i32s = np.zeros((128, 32), dtype=np.float32)
    i32s[np.arange(128), np.arange(128) % 32] = 1.0
    i128 = np.eye(128, dtype=np.float32)

    shared = dict(wih=wih, whh=whh, biasrow=biasrow, u1t=u1t, u2b=u2b,
                  i32s=i32s, i128=i128, onesrow=np.ones((1, 128), dtype=np.float32))

    in_maps = []
    for cidx in range(NC):
        sl = slice(cidx * BL, (cidx + 1) * BL)
        xc = x[:, sl, :]                                                # [T, 32, D]
        xT = np.ascontiguousarray(xc.reshape(T * BL, D).T, dtype=np.float32)
        ln = lengths[sl].astype(np.int64)
        t_start = np.maximum(ln - K, 0)
        t_k = t_start[:, None] + np.arange(K)[None, :]                  # [32, 4]
        valid = (t_k <= (ln[:, None] - 1))                              # [32, 4]
        tt = np.arange(T)
        mask = (tt[None, None, :] <= t_k[:, :, None]) & valid[:, :, None]  # [b, k, t]
        maskneg = np.where(mask, 0.0, NEG_INF).astype(np.float32)
        maskneg = np.ascontiguousarray(maskneg.reshape(BL, K * T))      # k-major cols
        in_maps.append(dict(shared, xT=xT, maskneg=maskneg,
                            valid=valid.astype(np.float32)))
    return in_maps


def kernel(**inputs) -> np.ndarray:
    inputs = {k: np.asarray(v) if not np.isscalar(v) else v for k, v in inputs.items()}
    in_maps = _host_prep(**inputs)
    if "nc" not in _cached:
        _cached["nc"] = _build_program()
    nc = _cached["nc"]
    res = run_bass_kernel_spmd(nc, in_maps, core_ids=list(range(NC)))
    outs = []
    for cidx in range(NC):
        o = res.results[cidx]["out"]                                    # [128, 512]
        outs.append(o.reshape(BL, K, H))
    return np.concatenate(outs, axis=0).astype(np.float32)              # [256, 4, 512]



# revision 46
# speedup vs baseline: 1.0330x; 1.0330x over previous
"""Trainium2 Bass kernel for DUPN-style LSTM + windowed-softmax attention pooling.

Math (per batch element b):
  LSTM over T=128 steps (torch gate order), hidden H=512, input D=256.
  a[t] = sigmoid(x[t]·u1 + h[t]·u2), u1 = (v1@A1)^T, u2 = (v1@A2)^T  (folded)
  out[b,k,:] = softmax-pooled sum of h[t] over window t <= t_k, for 4 slots.

Sharding: data-parallel over batch, 32 per core x 8 cores, weights replicated.

Transposed bf16 design (TRN2 cost model: matmul/elementwise cost ~ output
FREE size, independent of partition count — so gates live on partitions and
the 32-element batch on the free dim):
  - zT layout [128 gate-part, (step, g-chunk, batch)]: gate dim 128g+p,
    batch b. Host gate perm [i,f,o,g] => chunks 0-7=i,f / 8-11=o / 12-15=g,
    split into per-gate-group PSUM tiles (zif/zg/zo, bufs=2) so each
    activation's coalesced wait covers only its own group's matmuls.
  - Per step: 64 bf16 matmuls out [128, 32] (lhsT = W_hh block [128h, 128g],
    rhs = hT chunk [128h, 32b]), emitted i,f(c0,c1) -> i,f(c2,c3) -> g -> o;
    h-chunk halves first so the split tanh(c)/h tail overlaps the next
    step's first passes.
  - bias (K=16 indicator matmul) + xw (x@W_ih^T) accumulate directly into
    the pair's z PSUM tiles, emitted one pair ahead; PE drains them during
    the gate chain, before stalling on h(t-1).
  - Gate chain per step: sigma(i,f) [128,256] -> tanh(g) [128,128] on Act;
    tfc/tig/c-add on DVE in 64-col halves; tanh(c) + sigma(o)*tanh(c) in
    halves; h written bf16 straight into persistent hsT [128, t*128+32c+b]
    (next step's matmul operand and the pooling source — no transposes).
  - a_pre[t,b] = u1·x + u2·h via 6 contiguous out-[32,1] matmuls per step
    into a persistent [32,128] PSUM tile (interleaved accumulation groups
    get clobbered, so all 6 writes per column stay adjacent).
  - Post-loop: batched 4-slot softmax without max-subtract (logits are
    sigmoids in (0,1); masked -1e9 -> exp 0; eps guards invalid slots),
    then per-b: 4 PE transposes into one PSUM tile (emitted 3 b ahead of
    the in-order PE's pool matmuls), one [128,512] DVE copy (2x bf16 mode),
    pool matmuls grouped 4 b per [16,512] PSUM tile via zero-padded [T,16]
    weight slices => one Act copy + ONE output DMA per group (the SP
    sequencer's 650ns-per-DMA issue rate was the post-loop bottleneck).
"""
import sys

if "/opt/trn_rl_repo" not in sys.path:
    sys.path.insert(0, "/opt/trn_rl_repo")

import numpy as np
import ml_dtypes
import concourse.bass as bass
import concourse.bacc as bacc
import concourse.tile as tile
from concourse import mybir
from concourse.bass_utils import run_bass_kernel_spmd
from contextlib import ExitStack

F32 = mybir.dt.float32
BF16 = mybir.dt.bfloat16
AFT = mybir.ActivationFunctionType
ALU = mybir.AluOpType

T, BF, D, H, K, NC = 128, 256, 256, 512, 4, 8
BL = BF // NC          # 32 batch per core
G = 4 * H              # 2048
NEG_INF = -1e9
NP_ = T // 2           # 64 step-pairs

_cached = {}

# gate-group emission order: i,f first (feed sigma_if), then g (tanh_g),
# o last (sigma_o only gates the final h-mult)
GATE_ORDER = [0, 1, 2, 3, 4, 5, 6, 7, 12, 13, 14, 15, 8, 9, 10, 11]


def _build_program(t_steps=T):
    nc = bacc.Bacc()
    np_pairs = t_steps // 2
    # ---- DRAM I/O ----
    d_x = nc.declare_dram_parameter("xT", [D, t_steps * BL], BF16, isOutput=False)
    d_wih = nc.declare_dram_parameter("wih", [128, 2 * 16 * 128], BF16, isOutput=False)
    d_whh = nc.declare_dram_parameter("whh", [128, 4 * 16 * 128], BF16, isOutput=False)
    d_biasT = nc.declare_dram_parameter("biasT", [16, 128], BF16, isOutput=False)
    d_indif = nc.declare_dram_parameter("indif", [16, 512], BF16, isOutput=False)
    d_indg = nc.declare_dram_parameter("indg", [16, 256], BF16, isOutput=False)
    d_indo = nc.declare_dram_parameter("indo", [16, 256], BF16, isOutput=False)
    d_u1 = nc.declare_dram_parameter("u1m", [128, 2], BF16, isOutput=False)
    d_u2 = nc.declare_dram_parameter("u2m", [128, 4], BF16, isOutput=False)
    d_i128b = nc.declare_dram_parameter("i128b", [128, 128], BF16, isOutput=False)
    d_i32s = nc.declare_dram_parameter("i32s", [128, 32], F32, isOutput=False)
    d_maskneg = nc.declare_dram_parameter("maskneg", [BL, K * t_steps], F32, isOutput=False)
    d_valid = nc.declare_dram_parameter("valid", [BL, K], F32, isOutput=False)
    d_out = nc.declare_dram_parameter("out", [BL * K, H], F32, isOutput=True)

    with tile.TileContext(nc) as tc, ExitStack() as ctx:
        nv, ns, nt = nc.vector, nc.scalar, nc.tensor

        consts = ctx.enter_context(tc.tile_pool(name="consts", bufs=1))
        big = ctx.enter_context(tc.tile_pool(name="big", bufs=1))

        # ---- load constants ----
        wih_sb = consts.tile([128, 2 * 16 * 128], BF16, tag="wih")
        nc.sync.dma_start(wih_sb[:], d_wih[:])
        biasT_sb = consts.tile([16, 128], BF16, tag="biasT")
        nc.sync.dma_start(biasT_sb[:], d_biasT[:])
        indif_sb = consts.tile([16, 512], BF16, tag="indif")
        nc.sync.dma_start(indif_sb[:], d_indif[:])
        indg_sb = consts.tile([16, 256], BF16, tag="indg")
        nc.sync.dma_start(indg_sb[:], d_indg[:])
        indo_sb = consts.tile([16, 256], BF16, tag="indo")
        nc.sync.dma_start(indo_sb[:], d_indo[:])
        u1_sb = consts.tile([128, 2], BF16, tag="u1")
        nc.sync.dma_start(u1_sb[:], d_u1[:])
        u2_sb = consts.tile([128, 4], BF16, tag="u2")
        nc.sync.dma_start(u2_sb[:], d_u2[:])


        # ---- persistent state ----
        hsT = big.tile([128, t_steps * 128], BF16, tag="hsT")  # [p, t*128+c*32+b]
        cT = big.tile([128, 128], F32, tag="cT")               # [p, c*32+b]
        abp = big.tile([BL, t_steps], F32, tag="abp")

        # ---- loop pools ----
        loop_ctx = ExitStack()
        xp = loop_ctx.enter_context(tc.tile_pool(name="xp", bufs=6))
        gate_pool = loop_ctx.enter_context(tc.tile_pool(name="gate", bufs=3))
        tmp_pool = loop_ctx.enter_context(tc.tile_pool(name="tmp", bufs=3))
        ps_z = loop_ctx.enter_context(tc.tile_pool(name="ps_z", bufs=2, space="PSUM"))
        ps_a = loop_ctx.enter_context(tc.tile_pool(name="ps_a", bufs=1, space="PSUM"))

        pa = ps_a.tile([BL, t_steps], F32, tag="pa")

        xp_tiles = {}

        def dma_pair(P):
            xt = xp.tile([128, 128], BF16, tag="xpt", name=f"xpt{P}")
            nc.sync.dma_start(xt[:, 0:64], d_x[0:128, 64 * P:64 * P + 64])
            nc.sync.dma_start(xt[:, 64:128], d_x[128:256, 64 * P:64 * P + 64])
            xp_tiles[P] = xt

        def emit_a(t_):
            """a_pre[:, t_] = u1·x_t + u2·h_t, all 6 writes contiguous."""
            P_, s_ = divmod(t_, 2)
            xt = xp_tiles[P_]
            for d in range(2):
                nt.matmul(pa[:, t_:t_ + 1], xt[:, 64 * d + 32 * s_:64 * d + 32 * s_ + 32],
                          u1_sb[:, d:d + 1], start=(d == 0), stop=False)
            for c in range(4):
                nt.matmul(pa[:, t_:t_ + 1],
                          hsT[:, t_ * 128 + 32 * c:t_ * 128 + 32 * (c + 1)],
                          u2_sb[:, c:c + 1], start=False, stop=(c == 3))

        z_tiles = {}

        def zslice(tiles, g, s):
            """(tile, col) for gate-chunk g, step-in-pair s."""
            zif, zg_, zo_ = tiles
            if g < 8:
                return zif, 256 * s + 32 * g
            if g < 12:
                return zo_, 128 * s + 32 * (g - 8)
            return zg_, 128 * s + 32 * (g - 12)

        def emit_pair_mm(P):
            """bias + xw matmuls for pair P into fresh per-gate-group PSUM tiles."""
            zif = ps_z.tile([128, 512], F32, tag="zif", name=f"zif{P}")
            zg_ = ps_z.tile([128, 256], F32, tag="zg", name=f"zg{P}")
            zo_ = ps_z.tile([128, 256], F32, tag="zo", name=f"zo{P}")
            tiles = (zif, zg_, zo_)
            xt = xp_tiles[P]
            nt.matmul(zif[:], biasT_sb[:], indif_sb[:], start=True, stop=False)
            nt.matmul(zg_[:], biasT_sb[:], indg_sb[:], start=True, stop=False)
            nt.matmul(zo_[:], biasT_sb[:], indo_sb[:], start=True, stop=False)
            for s in range(2):
                t_ = 2 * P + s
                for d in range(2):
                    xs = xt[:, 64 * d + 32 * s:64 * d + 32 * s + 32]
                    for g in range(16):
                        # no h-passes only for t=0: its regions stop at d==1
                        ztile, col = zslice(tiles, g, s)
                        nt.matmul(ztile[:, col:col + 32],
                                  wih_sb[:, (16 * d + g) * 128:(16 * d + g + 1) * 128],
                                  xs, start=False,
                                  stop=(t_ == 0 and d == 1))
            z_tiles[P] = tiles
            return tiles

        for Pp in range(4):
            dma_pair(Pp)
        # whh is first needed at step 1; pooling consts at the post-loop —
        # their DMAs go after the x prefetch so step 0 starts early
        whh_sb = consts.tile([128, 4 * 16 * 128], BF16, tag="whh")
        nc.sync.dma_start(whh_sb[:], d_whh[:])
        i128b_sb = consts.tile([128, 128], BF16, tag="i128b")
        nc.sync.dma_start(i128b_sb[:], d_i128b[:])
        i32s_sb = consts.tile([128, 32], F32, tag="i32s")
        nc.sync.dma_start(i32s_sb[:], d_i32s[:])
        maskneg_sb = consts.tile([BL, K * t_steps], F32, tag="maskneg")
        nc.sync.dma_start(maskneg_sb[:], d_maskneg[:])
        valid_sb = consts.tile([BL, K], F32, tag="valid")
        nc.sync.dma_start(valid_sb[:], d_valid[:])
        emit_pair_mm(0)

        for t in range(t_steps):
            P, s = divmod(t, 2)
            zif, zg_, zo_ = z_tiles[P]
            # prefetch DMA + next pair's xw first: PE drains them during the
            # PREVIOUS step's gate chain, before stalling on h(t-1)
            if s == 0:
                if P + 4 < np_pairs:
                    dma_pair(P + 4)
                xp_tiles.pop(P - 2, None)
                if P + 1 < np_pairs:
                    emit_pair_mm(P + 1)
                z_tiles.pop(P - 1, None)
            # h-passes: contiguous on PE, gate-group order i,f -> g -> o;
            # per-gate-group PSUM tiles keep each activation's wait narrow
            hbase = (t - 1) * 128

            def hp(g, c):
                ztile, col = zslice(z_tiles[P], g, s)
                nt.matmul(ztile[:, col:col + 32],
                          whh_sb[:, (16 * c + g) * 128:(16 * c + g + 1) * 128],
                          hsT[:, hbase + 32 * c:hbase + 32 * (c + 1)],
                          start=False, stop=(c == 3))

            if t > 0:
                # i,f passes on h-chunks 0,1 first (ready before chunks 2,3)
                for g in range(8):
                    hp(g, 0)
                    hp(g, 1)
                for g in range(8):
                    hp(g, 2)
                    hp(g, 3)
                for g in [12, 13, 14, 15, 8, 9, 10, 11]:
                    for c in range(4):
                        hp(g, c)
            # gates
            sg = gate_pool.tile([128, 256], F32, tag="sg")
            ns.activation(sg[:], zif[:, 256 * s:256 * (s + 1)], AFT.Sigmoid)
            gg = gate_pool.tile([128, 128], F32, tag="gg")
            ns.activation(gg[:], zg_[:, 128 * s:128 * (s + 1)], AFT.Tanh)
            so = gate_pool.tile([128, 128], F32, tag="so")
            ns.activation(so[:], zo_[:, 128 * s:128 * (s + 1)], AFT.Sigmoid)
            if t == 0:
                nv.tensor_tensor(cT[:], sg[:, 0:128], gg[:], op=ALU.mult)
            else:
                tfc = tmp_pool.tile([128, 128], F32, tag="tfc")
                nv.tensor_tensor(tfc[:], sg[:, 128:256], cT[:], op=ALU.mult)
                tig = tmp_pool.tile([128, 128], F32, tag="tig")
                for hh in range(2):
                    sl = slice(64 * hh, 64 * (hh + 1))
                    nv.tensor_tensor(tig[:, sl], sg[:, 0:128][:, sl], gg[:, sl],
                                     op=ALU.mult)
                    nv.tensor_tensor(cT[:, sl], tfc[:, sl], tig[:, sl], op=ALU.add)
                # a_pre for step t-1 (h/x ready; fills PE idle in the chain)
                emit_a(t - 1)
            # tail in halves: h-chunks 0,1 land early so next step's c=0,1
            # matmul passes can start while the second half finishes
            tcs = tmp_pool.tile([128, 128], F32, tag="tcs")
            for hh in range(2):
                sl = slice(64 * hh, 64 * (hh + 1))
                ns.activation(tcs[:, sl], cT[:, sl], AFT.Tanh)
                nv.tensor_tensor(hsT[:, t * 128 + 64 * hh:t * 128 + 64 * (hh + 1)],
                                 so[:, sl], tcs[:, sl], op=ALU.mult)

        # final a_pre for last step
        emit_a(t_steps - 1)
        # a = sigmoid(a1 + a2), to SBUF before psum pools close
        ns.activation(abp[:], pa[:], AFT.Sigmoid)
        loop_ctx.close()

        # ---- post-loop: windowed softmax + pooling ----
        post = ctx.enter_context(tc.tile_pool(name="post", bufs=1))
        ps_t = ctx.enter_context(tc.tile_pool(name="ps_t", bufs=2, space="PSUM"))
        ps_pool = ctx.enter_context(tc.tile_pool(name="ps_pool", bufs=3, space="PSUM"))
        hsb_pool = ctx.enter_context(tc.tile_pool(name="hsb", bufs=4))
        stg_pool = ctx.enter_context(tc.tile_pool(name="stg", bufs=6))

        # hoist the first pooling transposes ahead of the softmax: they only
        # need hsT, and the in-order PE would otherwise park them behind the
        # wk transposes that wait for the whole softmax chain
        hsT_r = hsT[:].rearrange("p (t c b) -> p t c b", c=4, b=BL)
        pts = {}
        for b in range(3):
            pt = ps_t.tile([128, 512], BF16, tag="pt", bufs=4, name=f"pt{b}")
            for c in range(4):
                nt.transpose(pt[0:t_steps, 128 * c:128 * (c + 1)],
                             hsT_r[:, :, c, b], i128b_sb[:])
            pts[b] = pt

        # softmax per slot k -> wT [t, 4b+k] (bf16 for the pooling matmul)
        # batched over all 4 slots; logits are sigmoid outputs in (0,1) so
        # exp never overflows (no max-subtract) and masked -1e9 -> exp = 0
        wT = post.tile([t_steps, K * BL], BF16, tag="wT")
        scb = post.tile([BL, K * t_steps], F32, tag="scb")
        a_b = abp[:].rearrange("b (k t) -> b k t", k=1).broadcast_to([BL, K, t_steps])
        nv.tensor_tensor(scb[:].rearrange("b (k t) -> b k t", k=K), a_b,
                         maskneg_sb[:].rearrange("b (k t) -> b k t", k=K),
                         op=ALU.add)
        ekb = post.tile([BL, K * t_steps], F32, tag="ekb")
        ns.activation(ekb[:], scb[:], AFT.Exp)
        sk4 = post.tile([BL, K], F32, tag="sk4")
        nv.tensor_reduce(sk4[:], ekb[:].rearrange("b (k t) -> b k t", k=K),
                         axis=mybir.AxisListType.X, op=ALU.add)
        # all-masked (invalid) slots sum to 0 -> eps keeps 1/sum finite; the
        # valid-mask multiply below zeroes them exactly
        nv.tensor_scalar(out=sk4[:], in0=sk4[:], scalar1=1e-30, scalar2=None, op0=ALU.add)
        rk4 = post.tile([BL, K], F32, tag="rk4")
        nv.reciprocal(rk4[:], sk4[:])
        nv.tensor_tensor(rk4[:], rk4[:], valid_sb[:], op=ALU.mult)
        wkb = post.tile([BL, K * t_steps], F32, tag="wkb")
        r_b = rk4[:].rearrange("b (k o) -> b k o", o=1).broadcast_to([BL, K, t_steps])
        nv.tensor_tensor(wkb[:].rearrange("b (k t) -> b k t", k=K),
                         ekb[:].rearrange("b (k t) -> b k t", k=K), r_b,
                         op=ALU.mult)
        pwT = ps_t.tile([128, K * BL], F32, tag="pwT", bufs=1)
        for k in range(K):
            nt.transpose(pwT[0:t_steps, 32 * k:32 * (k + 1)],
                         wkb[:, t_steps * k:t_steps * (k + 1)], i32s_sb[0:32, :])
        nv.tensor_copy(wT[:].rearrange("t (b k) -> t b k", k=K),
                       pwT[0:t_steps, :].rearrange("t (k b) -> t b k", b=BL))
        # per-b zero-padded [T,16] weight slices for the grouped pool matmuls
        wTm_all = post.tile([t_steps, BL * 4 * K], BF16, tag="wTm_all")
        nv.memset(wTm_all[:], 0.0)

        # pooling: per b, rebuild hs_b [t, h] via 4 PE transposes into ONE
        # psum tile + one [128,512] copy (Act/DVE alternated). Pool matmuls
        # run in groups of 4 b's accumulating one [16,512] PSUM tile: each
        # matmul's weights are a zero-padded [T,16] slice (only cols 4j..4j+4
        # live), so cross-b terms vanish; one [16,512] copy + DMA per group.
        def emit_transposes(bb):
            ptn = ps_t.tile([128, 512], BF16, tag="pt", bufs=4, name=f"pt{bb}")
            for c in range(4):
                nt.transpose(ptn[0:t_steps, 128 * c:128 * (c + 1)],
                             hsT_r[:, :, c, bb], i128b_sb[:])
            pts[bb] = ptn

        for b in range(BL):
            # keep a 3-deep transpose lead so PE never waits behind pool mms
            if b + 3 < BL and (b + 3) not in pts:
                emit_transposes(b + 3)
            pt = pts.pop(b)
            hsb = hsb_pool.tile([t_steps, H], BF16, tag="hsb")
            nv.tensor_copy(hsb[:], pt[0:t_steps, :])
            j = b % 4
            wTm = wTm_all[:, 16 * b:16 * (b + 1)]
            nv.tensor_copy(wTm[:, 4 * j:4 * (j + 1)],
                           wT[0:t_steps, 4 * b:4 * (b + 1)])
            if j == 0:
                pp4 = ps_pool.tile([4 * K, H], F32, tag="pp4", name=f"pp4_{b}")
                pp4_hold = pp4
            else:
                pp4 = pp4_hold
            nt.matmul(pp4[:], wTm, hsb[:], start=(j == 0), stop=(j == 3))
            if j == 3:
                so4 = stg_pool.tile([4 * K, H], F32, tag="so4", name=f"so4_{b}")
                ns.copy(so4[:], pp4[:])
                nc.sync.dma_start(d_out[K * (b - 3):K * (b + 1), :], so4[:])

    nc.compile()
    return nc


def _host_prep(x, W_ih, W_hh, b_ih, b_hh, A1, A2, v1, lengths, label_len):
    assert int(label_len) == K
    BF16n = ml_dtypes.bfloat16
    perm = np.concatenate([np.arange(0, 512), np.arange(512, 1024),
                           np.arange(1536, 2048), np.arange(1024, 1536)])
    wih_f = np.ascontiguousarray(W_ih[perm].T, dtype=np.float32)   # [256, 2048]
    whh_f = np.ascontiguousarray(W_hh[perm].T, dtype=np.float32)   # [512, 2048]
    # blocks: wih[d-chunk, g-chunk] -> [128, (16d+g)*128 + j]
    wih = wih_f.reshape(2, 128, 16, 128).transpose(1, 0, 2, 3).reshape(128, -1)
    whh = whh_f.reshape(4, 128, 16, 128).transpose(1, 0, 2, 3).reshape(128, -1)
    bias = ((b_ih + b_hh)[perm]).astype(np.float32)
    biasT = bias.reshape(16, 128)                                  # [k, p]
    indif = np.zeros((16, 2, 8, 32), dtype=np.float32)
    indg = np.zeros((16, 2, 4, 32), dtype=np.float32)
    indo = np.zeros((16, 2, 4, 32), dtype=np.float32)
    for kk in range(8):
        indif[kk, :, kk, :] = 1.0
    for kk in range(4):
        indg[12 + kk, :, kk, :] = 1.0
        indo[8 + kk, :, kk, :] = 1.0
    indif = indif.reshape(16, 512)
    indg = indg.reshape(16, 256)
    indo = indo.reshape(16, 256)
    u1 = (v1 @ A1)[0].astype(np.float32)                           # [256]
    u2 = (v1 @ A2)[0].astype(np.float32)                           # [512]
    u1m = u1.reshape(2, 128).T.copy()                              # [128, 2]
    u2m = u2.reshape(4, 128).T.copy()                              # [128, 4]
    i128b = np.eye(128, dtype=np.float32)
    i32s = np.zeros((128, 32), dtype=np.float32)
    i32s[np.arange(128), np.arange(128) % 32] = 1.0

    shared = dict(wih=wih.astype(BF16n), whh=whh.astype(BF16n),
                  biasT=biasT.astype(BF16n), indif=indif.astype(BF16n),
                  indg=indg.astype(BF16n), indo=indo.astype(BF16n),
                  u1m=u1m.astype(BF16n), u2m=u2m.astype(BF16n),
                  i128b=i128b.astype(BF16n), i32s=i32s)

    in_maps = []
    for cidx in range(NC):
        sl = slice(cidx * BL, (cidx + 1) * BL)
        xc = x[:, sl, :]                                           # [T, 32, D]
        xT = np.ascontiguousarray(xc.reshape(T * BL, D).T, dtype=np.float32)
        ln = lengths[sl].astype(np.int64)
        t_start = np.maximum(ln - K, 0)
        t_k = t_start[:, None] + np.arange(K)[None, :]             # [32, 4]
        valid = (t_k <= (ln[:, None] - 1))                         # [32, 4]
        tt = np.arange(T)
        mask = (tt[None, None, :] <= t_k[:, :, None]) & valid[:, :, None]
        maskneg = np.where(mask, 0.0, NEG_INF).astype(np.float32)
        maskneg = np.ascontiguousarray(maskneg.reshape(BL, K * T))
        in_maps.append(dict(shared, xT=xT.astype(BF16n), maskneg=maskneg,
                            valid=valid.astype(np.float32)))
    return in_maps


def kernel(**inputs) -> np.ndarray:
    inputs = {k: np.asarray(v) if not np.isscalar(v) else v for k, v in inputs.items()}
    in_maps = _host_prep(**inputs)
    if "nc" not in _cached:
        _cached["nc"] = _build_program()
    nc = _cached["nc"]
    res = run_bass_kernel_spmd(nc, in_maps, core_ids=list(range(NC)))
    outs = []
    for cidx in range(NC):
        o = res.results[cidx]["out"]                               # [128, 512]
        outs.append(o.reshape(BL, K, H))
    return np.concatenate(outs, axis=0).astype(np.float32)         # [256, 4, 512]


# revision 48
# speedup vs baseline: 1.0634x; 1.0294x over previous
"""Trainium2 Bass kernel for DUPN-style LSTM + windowed-softmax attention pooling.

Math (per batch element b):
  LSTM over T=128 steps (torch gate order), hidden H=512, input D=256.
  a[t] = sigmoid(x[t]·u1 + h[t]·u2), u1 = (v1@A1)^T, u2 = (v1@A2)^T  (folded)
  out[b,k,:] = softmax-pooled sum of h[t] over window t <= t_k, for 4 slots.

Sharding: data-parallel over batch, 32 per core x 8 cores, weights replicated.

Transposed bf16 design (TRN2 cost model: matmul/elementwise cost ~ output
FREE size, independent of partition count — so gates live on partitions and
the 32-element batch on the free dim):
  - zT layout [128 gate-part, (step, g-chunk, batch)]: gate dim 128g+p,
    batch b. Host gate perm [i,f,o,g] => chunks 0-7=i,f / 8-11=o / 12-15=g,
    split into per-gate-group PSUM tiles (zif/zg/zo, bufs=2) so each
    activation's coalesced wait covers only its own group's matmuls.
  - Per step: 64 bf16 matmuls out [128, 32] (lhsT = W_hh block [128h, 128g],
    rhs = hT chunk [128h, 32b]), emitted i,f(c0,c1) -> i,f(c2,c3) -> g -> o;
    h-chunk halves first so the split tanh(c)/h tail overlaps the next
    step's first passes.
  - bias (K=16 indicator matmul) + xw (x@W_ih^T) accumulate directly into
    the pair's z PSUM tiles, emitted one pair ahead; PE drains them during
    the gate chain, before stalling on h(t-1).
  - Gate chain per step: sigma(i,f) [128,256] -> tanh(g) [128,128] on Act;
    tfc/tig/c-add on DVE in 64-col halves; tanh(c) + sigma(o)*tanh(c) in
    halves; h written bf16 straight into persistent hsT [128, t*128+32c+b]
    (next step's matmul operand and the pooling source — no transposes).
  - a_pre[t,b] = u1·x + u2·h via 6 contiguous out-[32,1] matmuls per step
    into a persistent [32,128] PSUM tile (interleaved accumulation groups
    get clobbered, so all 6 writes per column stay adjacent).
  - Post-loop: batched 4-slot softmax without max-subtract (logits are
    sigmoids in (0,1); masked -1e9 -> exp 0; eps guards invalid slots),
    then per-b: 4 PE transposes into one PSUM tile (emitted 3 b ahead of
    the in-order PE's pool matmuls), one [128,512] DVE copy (2x bf16 mode),
    pool matmuls grouped 4 b per [16,512] PSUM tile via zero-padded [T,16]
    weight slices => one Act copy + ONE output DMA per group (the SP
    sequencer's 650ns-per-DMA issue rate was the post-loop bottleneck).
"""
import sys

if "/opt/trn_rl_repo" not in sys.path:
    sys.path.insert(0, "/opt/trn_rl_repo")

import numpy as np
import ml_dtypes
import concourse.bass as bass
import concourse.bacc as bacc
import concourse.tile as tile
from concourse import mybir
from concourse.bass_utils import run_bass_kernel_spmd
from contextlib import ExitStack

F32 = mybir.dt.float32
BF16 = mybir.dt.bfloat16
AFT = mybir.ActivationFunctionType
ALU = mybir.AluOpType

T, BF, D, H, K, NC = 128, 256, 256, 512, 4, 8
BL = BF // NC          # 32 batch per core
G = 4 * H              # 2048
NEG_INF = -1e9
NP_ = T // 2           # 64 step-pairs

_cached = {}

# gate-group emission order: i,f first (feed sigma_if), then g (tanh_g),
# o last (sigma_o only gates the final h-mult)
GATE_ORDER = [0, 1, 2, 3, 4, 5, 6, 7, 12, 13, 14, 15, 8, 9, 10, 11]


def _build_program(t_steps=T):
    nc = bacc.Bacc()
    np_pairs = t_steps // 2
    # ---- DRAM I/O ----
    d_x = nc.declare_dram_parameter("xT", [D, t_steps * BL], BF16, isOutput=False)
    d_wih = nc.declare_dram_parameter("wih", [128, 2 * 16 * 128], BF16, isOutput=False)
    d_whh = nc.declare_dram_parameter("whh", [128, 4 * 16 * 128], BF16, isOutput=False)
    d_pack16 = nc.declare_dram_parameter("pack16", [16, 1152], BF16, isOutput=False)
    d_u12 = nc.declare_dram_parameter("u12", [128, 6], BF16, isOutput=False)
    d_i128b = nc.declare_dram_parameter("i128b", [128, 128], BF16, isOutput=False)
    d_i32s = nc.declare_dram_parameter("i32s", [128, 32], F32, isOutput=False)
    d_maskneg = nc.declare_dram_parameter("maskneg", [BL, K * t_steps], F32, isOutput=False)
    d_valid = nc.declare_dram_parameter("valid", [BL, K], F32, isOutput=False)
    d_out = nc.declare_dram_parameter("out", [BL * K, H], F32, isOutput=True)

    with tile.TileContext(nc) as tc, ExitStack() as ctx:
        nv, ns, nt = nc.vector, nc.scalar, nc.tensor

        consts = ctx.enter_context(tc.tile_pool(name="consts", bufs=1))
        big = ctx.enter_context(tc.tile_pool(name="big", bufs=1))

        # ---- load constants ----
        wih_sb = consts.tile([128, 2 * 16 * 128], BF16, tag="wih")
        nc.sync.dma_start(wih_sb[:], d_wih[:])
        pack16_sb = consts.tile([16, 1152], BF16, tag="pack16")
        nc.sync.dma_start(pack16_sb[:], d_pack16[:])
        biasT_sb = pack16_sb[:, 0:128]
        indif_sb = pack16_sb[:, 128:640]
        indg_sb = pack16_sb[:, 640:896]
        indo_sb = pack16_sb[:, 896:1152]
        u12_sb = consts.tile([128, 6], BF16, tag="u12")
        nc.sync.dma_start(u12_sb[:], d_u12[:])
        u1_sb = u12_sb[:, 0:2]
        u2_sb = u12_sb[:, 2:6]


        # ---- persistent state ----
        hsT = big.tile([128, t_steps * 128], BF16, tag="hsT")  # [p, t*128+c*32+b]
        cT = big.tile([128, 128], F32, tag="cT")               # [p, c*32+b]
        abp = big.tile([BL, t_steps], F32, tag="abp")

        # ---- loop pools ----
        loop_ctx = ExitStack()
        xp = loop_ctx.enter_context(tc.tile_pool(name="xp", bufs=6))
        gate_pool = loop_ctx.enter_context(tc.tile_pool(name="gate", bufs=3))
        tmp_pool = loop_ctx.enter_context(tc.tile_pool(name="tmp", bufs=3))
        ps_z = loop_ctx.enter_context(tc.tile_pool(name="ps_z", bufs=2, space="PSUM"))
        ps_a = loop_ctx.enter_context(tc.tile_pool(name="ps_a", bufs=1, space="PSUM"))

        pa = ps_a.tile([BL, t_steps], F32, tag="pa")

        xp_tiles = {}

        def dma_pair(P):
            xt = xp.tile([128, 128], BF16, tag="xpt", name=f"xpt{P}")
            nc.sync.dma_start(
                xt[:].rearrange("p (d c) -> p d c", d=2),
                d_x[:].rearrange("(d p) c -> p d c", d=2)[:, :, 64 * P:64 * P + 64])
            xp_tiles[P] = xt

        def emit_a(t_):
            """a_pre[:, t_] = u1·x_t + u2·h_t, all 6 writes contiguous."""
            P_, s_ = divmod(t_, 2)
            xt = xp_tiles[P_]
            for d in range(2):
                nt.matmul(pa[:, t_:t_ + 1], xt[:, 64 * d + 32 * s_:64 * d + 32 * s_ + 32],
                          u1_sb[:, d:d + 1], start=(d == 0), stop=False)
            for c in range(4):
                nt.matmul(pa[:, t_:t_ + 1],
                          hsT[:, t_ * 128 + 32 * c:t_ * 128 + 32 * (c + 1)],
                          u2_sb[:, c:c + 1], start=False, stop=(c == 3))

        z_tiles = {}

        def zslice(tiles, g, s):
            """(tile, col) for gate-chunk g, step-in-pair s."""
            zif, zg_, zo_ = tiles
            if g < 8:
                return zif, 256 * s + 32 * g
            if g < 12:
                return zo_, 128 * s + 32 * (g - 8)
            return zg_, 128 * s + 32 * (g - 12)

        def emit_pair_mm(P):
            """bias + xw matmuls for pair P into fresh per-gate-group PSUM tiles."""
            zif = ps_z.tile([128, 512], F32, tag="zif", name=f"zif{P}")
            zg_ = ps_z.tile([128, 256], F32, tag="zg", name=f"zg{P}")
            zo_ = ps_z.tile([128, 256], F32, tag="zo", name=f"zo{P}")
            tiles = (zif, zg_, zo_)
            xt = xp_tiles[P]
            nt.matmul(zif[:], biasT_sb, indif_sb, start=True, stop=False)
            nt.matmul(zg_[:], biasT_sb, indg_sb, start=True, stop=False)
            nt.matmul(zo_[:], biasT_sb, indo_sb, start=True, stop=False)
            for s in range(2):
                t_ = 2 * P + s
                for d in range(2):
                    xs = xt[:, 64 * d + 32 * s:64 * d + 32 * s + 32]
                    for g in range(16):
                        # no h-passes only for t=0: its regions stop at d==1
                        ztile, col = zslice(tiles, g, s)
                        nt.matmul(ztile[:, col:col + 32],
                                  wih_sb[:, (16 * d + g) * 128:(16 * d + g + 1) * 128],
                                  xs, start=False,
                                  stop=(t_ == 0 and d == 1))
            z_tiles[P] = tiles
            return tiles

        for Pp in range(4):
            dma_pair(Pp)
        # whh is first needed at step 1; pooling consts at the post-loop —
        # their DMAs go after the x prefetch so step 0 starts early
        whh_sb = consts.tile([128, 4 * 16 * 128], BF16, tag="whh")
        nc.sync.dma_start(whh_sb[:], d_whh[:])
        i128b_sb = consts.tile([128, 128], BF16, tag="i128b")
        nc.sync.dma_start(i128b_sb[:], d_i128b[:])
        i32s_sb = consts.tile([128, 32], F32, tag="i32s")
        nc.sync.dma_start(i32s_sb[:], d_i32s[:])
        maskneg_sb = consts.tile([BL, K * t_steps], F32, tag="maskneg")
        nc.sync.dma_start(maskneg_sb[:], d_maskneg[:])
        valid_sb = consts.tile([BL, K], F32, tag="valid")
        nc.sync.dma_start(valid_sb[:], d_valid[:])
        emit_pair_mm(0)

        for t in range(t_steps):
            P, s = divmod(t, 2)
            zif, zg_, zo_ = z_tiles[P]
            # prefetch DMA + next pair's xw first: PE drains them during the
            # PREVIOUS step's gate chain, before stalling on h(t-1)
            if s == 0:
                if P + 4 < np_pairs:
                    dma_pair(P + 4)
                xp_tiles.pop(P - 2, None)
                if P + 1 < np_pairs:
                    emit_pair_mm(P + 1)
                z_tiles.pop(P - 1, None)
            # h-passes: contiguous on PE, gate-group order i,f -> g -> o;
            # per-gate-group PSUM tiles keep each activation's wait narrow
            hbase = (t - 1) * 128

            def hp(g, c):
                ztile, col = zslice(z_tiles[P], g, s)
                nt.matmul(ztile[:, col:col + 32],
                          whh_sb[:, (16 * c + g) * 128:(16 * c + g + 1) * 128],
                          hsT[:, hbase + 32 * c:hbase + 32 * (c + 1)],
                          start=False, stop=(c == 3))

            if t > 0:
                # i,f passes on h-chunks 0,1 first (ready before chunks 2,3)
                for g in range(8):
                    hp(g, 0)
                    hp(g, 1)
                for g in range(8):
                    hp(g, 2)
                    hp(g, 3)
                for g in [12, 13, 14, 15, 8, 9, 10, 11]:
                    for c in range(4):
                        hp(g, c)
            # gates
            sg = gate_pool.tile([128, 256], F32, tag="sg")
            ns.activation(sg[:], zif[:, 256 * s:256 * (s + 1)], AFT.Sigmoid)
            gg = gate_pool.tile([128, 128], F32, tag="gg")
            ns.activation(gg[:], zg_[:, 128 * s:128 * (s + 1)], AFT.Tanh)
            so = gate_pool.tile([128, 128], F32, tag="so")
            ns.activation(so[:], zo_[:, 128 * s:128 * (s + 1)], AFT.Sigmoid)
            if t == 0:
                nv.tensor_tensor(cT[:], sg[:, 0:128], gg[:], op=ALU.mult)
            else:
                tfc = tmp_pool.tile([128, 128], F32, tag="tfc")
                nv.tensor_tensor(tfc[:], sg[:, 128:256], cT[:], op=ALU.mult)
                tig = tmp_pool.tile([128, 128], F32, tag="tig")
                for hh in range(2):
                    sl = slice(64 * hh, 64 * (hh + 1))
                    nv.tensor_tensor(tig[:, sl], sg[:, 0:128][:, sl], gg[:, sl],
                                     op=ALU.mult)
                    nv.tensor_tensor(cT[:, sl], tfc[:, sl], tig[:, sl], op=ALU.add)
                # a_pre for step t-1 (h/x ready; fills PE idle in the chain)
                emit_a(t - 1)
            # tail in halves: h-chunks 0,1 land early so next step's c=0,1
            # matmul passes can start while the second half finishes
            tcs = tmp_pool.tile([128, 128], F32, tag="tcs")
            for hh in range(2):
                sl = slice(64 * hh, 64 * (hh + 1))
                ns.activation(tcs[:, sl], cT[:, sl], AFT.Tanh)
                nv.tensor_tensor(hsT[:, t * 128 + 64 * hh:t * 128 + 64 * (hh + 1)],
                                 so[:, sl], tcs[:, sl], op=ALU.mult)

        # final a_pre for last step
        emit_a(t_steps - 1)
        # a = sigmoid(a1 + a2), to SBUF before psum pools close
        ns.activation(abp[:], pa[:], AFT.Sigmoid)
        loop_ctx.close()

        # ---- post-loop: windowed softmax + pooling ----
        post = ctx.enter_context(tc.tile_pool(name="post", bufs=1))
        ps_t = ctx.enter_context(tc.tile_pool(name="ps_t", bufs=2, space="PSUM"))
        ps_pool = ctx.enter_context(tc.tile_pool(name="ps_pool", bufs=3, space="PSUM"))
        hsb_pool = ctx.enter_context(tc.tile_pool(name="hsb", bufs=4))
        stg_pool = ctx.enter_context(tc.tile_pool(name="stg", bufs=6))

        # hoist the first pooling transposes ahead of the softmax: they only
        # need hsT, and the in-order PE would otherwise park them behind the
        # wk transposes that wait for the whole softmax chain
        hsT_r = hsT[:].rearrange("p (t c b) -> p t c b", c=4, b=BL)
        pts = {}
        for b in range(3):
            pt = ps_t.tile([128, 512], BF16, tag="pt", bufs=4, name=f"pt{b}")
            for c in range(4):
                nt.transpose(pt[0:t_steps, 128 * c:128 * (c + 1)],
                             hsT_r[:, :, c, b], i128b_sb[:])
            pts[b] = pt

        # softmax per slot k -> wT [t, 4b+k] (bf16 for the pooling matmul)
        # batched over all 4 slots; logits are sigmoid outputs in (0,1) so
        # exp never overflows (no max-subtract) and masked -1e9 -> exp = 0
        wT = post.tile([t_steps, K * BL], BF16, tag="wT")
        scb = post.tile([BL, K * t_steps], F32, tag="scb")
        a_b = abp[:].rearrange("b (k t) -> b k t", k=1).broadcast_to([BL, K, t_steps])
        nv.tensor_tensor(scb[:].rearrange("b (k t) -> b k t", k=K), a_b,
                         maskneg_sb[:].rearrange("b (k t) -> b k t", k=K),
                         op=ALU.add)
        ekb = post.tile([BL, K * t_steps], F32, tag="ekb")
        ns.activation(ekb[:], scb[:], AFT.Exp)
        sk4 = post.tile([BL, K], F32, tag="sk4")
        nv.tensor_reduce(sk4[:], ekb[:].rearrange("b (k t) -> b k t", k=K),
                         axis=mybir.AxisListType.X, op=ALU.add)
        # all-masked (invalid) slots sum to 0 -> eps keeps 1/sum finite; the
        # valid-mask multiply below zeroes them exactly
        nv.tensor_scalar(out=sk4[:], in0=sk4[:], scalar1=1e-30, scalar2=None, op0=ALU.add)
        rk4 = post.tile([BL, K], F32, tag="rk4")
        nv.reciprocal(rk4[:], sk4[:])
        nv.tensor_tensor(rk4[:], rk4[:], valid_sb[:], op=ALU.mult)
        wkb = post.tile([BL, K * t_steps], F32, tag="wkb")
        r_b = rk4[:].rearrange("b (k o) -> b k o", o=1).broadcast_to([BL, K, t_steps])
        nv.tensor_tensor(wkb[:].rearrange("b (k t) -> b k t", k=K),
                         ekb[:].rearrange("b (k t) -> b k t", k=K), r_b,
                         op=ALU.mult)
        pwT = ps_t.tile([128, K * BL], F32, tag="pwT", bufs=1)
        for k in range(K):
            nt.transpose(pwT[0:t_steps, 32 * k:32 * (k + 1)],
                         wkb[:, t_steps * k:t_steps * (k + 1)], i32s_sb[0:32, :])
        nv.tensor_copy(wT[:].rearrange("t (b k) -> t b k", k=K),
                       pwT[0:t_steps, :].rearrange("t (k b) -> t b k", b=BL))
        # per-b zero-padded [T,16] weight slices for the grouped pool matmuls
        wTm_all = post.tile([t_steps, BL * 4 * K], BF16, tag="wTm_all")
        nv.memset(wTm_all[:], 0.0)

        # pooling: per b, rebuild hs_b [t, h] via 4 PE transposes into ONE
        # psum tile + one [128,512] copy (Act/DVE alternated). Pool matmuls
        # run in groups of 4 b's accumulating one [16,512] PSUM tile: each
        # matmul's weights are a zero-padded [T,16] slice (only cols 4j..4j+4
        # live), so cross-b terms vanish; one [16,512] copy + DMA per group.
        def emit_transposes(bb):
            ptn = ps_t.tile([128, 512], BF16, tag="pt", bufs=4, name=f"pt{bb}")
            for c in range(4):
                nt.transpose(ptn[0:t_steps, 128 * c:128 * (c + 1)],
                             hsT_r[:, :, c, bb], i128b_sb[:])
            pts[bb] = ptn

        for b in range(BL):
            # keep a 3-deep transpose lead so PE never waits behind pool mms
            if b + 3 < BL and (b + 3) not in pts:
                emit_transposes(b + 3)
            pt = pts.pop(b)
            hsb = hsb_pool.tile([t_steps, H], BF16, tag="hsb")
            if b % 4 == 0:
                ns.copy(hsb[:], pt[0:t_steps, :])
            else:
                nv.tensor_copy(hsb[:], pt[0:t_steps, :])
            j = b % 4
            wTm = wTm_all[:, 16 * b:16 * (b + 1)]
            nv.tensor_copy(wTm[:, 4 * j:4 * (j + 1)],
                           wT[0:t_steps, 4 * b:4 * (b + 1)])
            if j == 0:
                pp4 = ps_pool.tile([4 * K, H], F32, tag="pp4", name=f"pp4_{b}")
                pp4_hold = pp4
            else:
                pp4 = pp4_hold
            nt.matmul(pp4[:], wTm, hsb[:], start=(j == 0), stop=(j == 3))
            if j == 3:
                so4 = stg_pool.tile([4 * K, H], F32, tag="so4", name=f"so4_{b}")
                ns.copy(so4[:], pp4[:])
                nc.sync.dma_start(d_out[K * (b - 3):K * (b + 1), :], so4[:])

    nc.compile()
    return nc


def _host_prep(x, W_ih, W_hh, b_ih, b_hh, A1, A2, v1, lengths, label_len):
    assert int(label_len) == K
    BF16n = ml_dtypes.bfloat16
    perm = np.concatenate([np.arange(0, 512), np.arange(512, 1024),
                           np.arange(1536, 2048), np.arange(1024, 1536)])
    wih_f = np.ascontiguousarray(W_ih[perm].T, dtype=np.float32)   # [256, 2048]
    whh_f = np.ascontiguousarray(W_hh[perm].T, dtype=np.float32)   # [512, 2048]
    # blocks: wih[d-chunk, g-chunk] -> [128, (16d+g)*128 + j]
    wih = wih_f.reshape(2, 128, 16, 128).transpose(1, 0, 2, 3).reshape(128, -1)
    whh = whh_f.reshape(4, 128, 16, 128).transpose(1, 0, 2, 3).reshape(128, -1)
    bias = ((b_ih + b_hh)[perm]).astype(np.float32)
    biasT = bias.reshape(16, 128)                                  # [k, p]
    indif = np.zeros((16, 2, 8, 32), dtype=np.float32)
    indg = np.zeros((16, 2, 4, 32), dtype=np.float32)
    indo = np.zeros((16, 2, 4, 32), dtype=np.float32)
    for kk in range(8):
        indif[kk, :, kk, :] = 1.0
    for kk in range(4):
        indg[12 + kk, :, kk, :] = 1.0
        indo[8 + kk, :, kk, :] = 1.0
    indif = indif.reshape(16, 512)
    indg = indg.reshape(16, 256)
    indo = indo.reshape(16, 256)
    u1 = (v1 @ A1)[0].astype(np.float32)                           # [256]
    u2 = (v1 @ A2)[0].astype(np.float32)                           # [512]
    u1m = u1.reshape(2, 128).T.copy()                              # [128, 2]
    u2m = u2.reshape(4, 128).T.copy()                              # [128, 4]
    i128b = np.eye(128, dtype=np.float32)
    i32s = np.zeros((128, 32), dtype=np.float32)
    i32s[np.arange(128), np.arange(128) % 32] = 1.0

    pack16 = np.concatenate([biasT, indif, indg, indo], axis=1)   # [16, 1152]
    u12 = np.concatenate([u1m, u2m], axis=1)                      # [128, 6]
    shared = dict(wih=wih.astype(BF16n), whh=whh.astype(BF16n),
                  pack16=pack16.astype(BF16n), u12=u12.astype(BF16n),
                  i128b=i128b.astype(BF16n), i32s=i32s)

    in_maps = []
    for cidx in range(NC):
        sl = slice(cidx * BL, (cidx + 1) * BL)
        xc = x[:, sl, :]                                           # [T, 32, D]
        xT = np.ascontiguousarray(xc.reshape(T * BL, D).T, dtype=np.float32)
        ln = lengths[sl].astype(np.int64)
        t_start = np.maximum(ln - K, 0)
        t_k = t_start[:, None] + np.arange(K)[None, :]             # [32, 4]
        valid = (t_k <= (ln[:, None] - 1))                         # [32, 4]
        tt = np.arange(T)
        mask = (tt[None, None, :] <= t_k[:, :, None]) & valid[:, :, None]
        maskneg = np.where(mask, 0.0, NEG_INF).astype(np.float32)
        maskneg = np.ascontiguousarray(maskneg.reshape(BL, K * T))
        in_maps.append(dict(shared, xT=xT.astype(BF16n), maskneg=maskneg,
                            valid=valid.astype(np.float32)))
    return in_maps


def kernel(**inputs) -> np.ndarray:
    inputs = {k: np.asarray(v) if not np.isscalar(v) else v for k, v in inputs.items()}
    in_maps = _host_prep(**inputs)
    if "nc" not in _cached:
        _cached["nc"] = _build_program()
    nc = _cached["nc"]
    res = run_bass_kernel_spmd(nc, in_maps, core_ids=list(range(NC)))
    outs = []
    for cidx in range(NC):
        o = res.results[cidx]["out"]                               # [128, 512]
        outs.append(o.reshape(BL, K, H))
    return np.concatenate(outs, axis=0).astype(np.float32)         # [256, 4, 512]


# revision 52
# speedup vs baseline: 1.1489x; 1.0804x over previous
"""Trainium2 Bass kernel for DUPN-style LSTM + windowed-softmax attention pooling.

Math (per batch element b):
  LSTM over T=128 steps (torch gate order), hidden H=512, input D=256.
  a[t] = sigmoid(x[t]·u1 + h[t]·u2), u1 = (v1@A1)^T, u2 = (v1@A2)^T  (folded)
  out[b,k,:] = softmax-pooled sum of h[t] over window t <= t_k, for 4 slots.

Sharding: data-parallel over batch, 32 per core x 8 cores, weights replicated.

Transposed bf16 design (TRN2 cost model: matmul/elementwise cost ~ output
FREE size, independent of partition count — so gates live on partitions and
the 32-element batch on the free dim):
  - zT layout [128 gate-part, (step, g-chunk, batch)]: gate dim 128g+p,
    batch b. Host gate perm [i,f,o,g] => chunks 0-7=i,f / 8-11=o / 12-15=g,
    split into per-gate-group PSUM tiles (zif/zg/zo, bufs=2) so each
    activation's coalesced wait covers only its own group's matmuls.
  - Per step: 64 bf16 matmuls out [128, 32] (lhsT = W_hh block [128h, 128g],
    rhs = hT chunk [128h, 32b]), emitted i,f(c0,c1) -> i,f(c2,c3) -> g -> o;
    h-chunk halves first so the split tanh(c)/h tail overlaps the next
    step's first passes.
  - bias (K=16 indicator matmul) + xw (x@W_ih^T) accumulate directly into
    the pair's z PSUM tiles, emitted on the preceding odd iteration; the
    in-order PE drains them inside the gate-chain idle window. (The loop is
    chain-bound: PE placement is pure slack — verified by a timing-identical
    restructure.)
  - Gate chain per step: sigma(i,f) [128,256] -> tanh(g) [128,128] on Act;
    tfc/tig/c-add on DVE in 64-col halves; tanh(c) + sigma(o)*tanh(c) in
    halves; h written bf16 straight into persistent hsT [128, t*128+32c+b]
    (next step's matmul operand and the pooling source — no transposes).
  - a_pre[t,b] = u1·x + u2·h via 6 contiguous out-[32,1] matmuls per step
    into a persistent [32,128] PSUM tile (interleaved accumulation groups
    get clobbered, so all 6 writes per column stay adjacent).
  - Post-loop: batched 4-slot softmax without max-subtract (logits are
    sigmoids in (0,1); masked -1e9 -> exp 0; eps guards invalid slots),
    then per-b: 4 PE transposes into one PSUM tile (emitted 3 b ahead of
    the in-order PE's pool matmuls), one [128,512] DVE copy (2x bf16 mode),
    pool matmuls grouped 4 b per [16,512] PSUM tile via zero-padded [T,16]
    weight slices => one Act copy + ONE output DMA per group (the SP
    sequencer's 650ns-per-DMA issue rate was the post-loop bottleneck).
"""
import sys

if "/opt/trn_rl_repo" not in sys.path:
    sys.path.insert(0, "/opt/trn_rl_repo")

import numpy as np
import ml_dtypes
import concourse.bass as bass
import concourse.bacc as bacc
import concourse.tile as tile
from concourse import mybir
from concourse.bass_utils import run_bass_kernel_spmd
from contextlib import ExitStack

F32 = mybir.dt.float32
BF16 = mybir.dt.bfloat16
AFT = mybir.ActivationFunctionType
ALU = mybir.AluOpType

T, BF, D, H, K, NC = 128, 256, 256, 512, 4, 8
BL = BF // NC          # 32 batch per core
G = 4 * H              # 2048
NEG_INF = -1e9
NP_ = T // 2           # 64 step-pairs

_cached = {}

# gate-group emission order: i,f first (feed sigma_if), then g (tanh_g),
# o last (sigma_o only gates the final h-mult)
GATE_ORDER = [0, 1, 2, 3, 4, 5, 6, 7, 12, 13, 14, 15, 8, 9, 10, 11]


def _build_program(t_steps=T):
    nc = bacc.Bacc()
    np_pairs = t_steps // 2
    # ---- DRAM I/O ----
    d_x = nc.declare_dram_parameter("xT", [D, t_steps * BL], BF16, isOutput=False)
    d_wih = nc.declare_dram_parameter("wih", [128, 2 * 16 * 128], BF16, isOutput=False)
    d_whh = nc.declare_dram_parameter("whh", [128, 4 * 16 * 128], BF16, isOutput=False)
    d_pack16 = nc.declare_dram_parameter("pack16", [16, 1152], BF16, isOutput=False)
    d_u12 = nc.declare_dram_parameter("u12", [128, 6], BF16, isOutput=False)
    d_i128b = nc.declare_dram_parameter("i128b", [128, 128], BF16, isOutput=False)
    d_i32s = nc.declare_dram_parameter("i32s", [128, 32], F32, isOutput=False)
    d_maskneg = nc.declare_dram_parameter("maskneg", [BL, K * t_steps], F32, isOutput=False)
    d_valid = nc.declare_dram_parameter("valid", [BL, K], F32, isOutput=False)
    d_out = nc.declare_dram_parameter("out", [BL * K, H], F32, isOutput=True)

    with tile.TileContext(nc) as tc, ExitStack() as ctx:
        nv, ns, nt = nc.vector, nc.scalar, nc.tensor

        consts = ctx.enter_context(tc.tile_pool(name="consts", bufs=1))
        big = ctx.enter_context(tc.tile_pool(name="big", bufs=1))

        # ---- load constants ----
        wih_sb = consts.tile([128, 2 * 16 * 128], BF16, tag="wih")
        nc.sync.dma_start(wih_sb[:], d_wih[:])
        pack16_sb = consts.tile([16, 1152], BF16, tag="pack16")
        nc.sync.dma_start(pack16_sb[:], d_pack16[:])
        biasT_sb = pack16_sb[:, 0:128]
        indif_sb = pack16_sb[:, 128:640]
        indg_sb = pack16_sb[:, 640:896]
        indo_sb = pack16_sb[:, 896:1152]
        u12_sb = consts.tile([128, 6], BF16, tag="u12")
        nc.sync.dma_start(u12_sb[:], d_u12[:])
        u1_sb = u12_sb[:, 0:2]
        u2_sb = u12_sb[:, 2:6]


        # ---- persistent state ----
        hsT = big.tile([128, t_steps * 128], BF16, tag="hsT")  # [p, t*128+c*32+b]
        cT = big.tile([128, 128], F32, tag="cT")               # [p, c*32+b]
        abp = big.tile([BL, t_steps], F32, tag="abp")

        # ---- loop pools ----
        loop_ctx = ExitStack()
        xp = loop_ctx.enter_context(tc.tile_pool(name="xp", bufs=6))
        gate_pool = loop_ctx.enter_context(tc.tile_pool(name="gate", bufs=3))
        tmp_pool = loop_ctx.enter_context(tc.tile_pool(name="tmp", bufs=3))
        ps_z = loop_ctx.enter_context(tc.tile_pool(name="ps_z", bufs=2, space="PSUM"))
        ps_a = loop_ctx.enter_context(tc.tile_pool(name="ps_a", bufs=1, space="PSUM"))

        pa = ps_a.tile([BL, t_steps], F32, tag="pa")

        xp_tiles = {}

        def dma_pair(P):
            xt = xp.tile([128, 128], BF16, tag="xpt", name=f"xpt{P}")
            nc.sync.dma_start(
                xt[:].rearrange("p (d c) -> p d c", d=2),
                d_x[:].rearrange("(d p) c -> p d c", d=2)[:, :, 64 * P:64 * P + 64])
            xp_tiles[P] = xt

        def emit_a(t_):
            """a_pre[:, t_] = u1·x_t + u2·h_t, all 6 writes contiguous."""
            P_, s_ = divmod(t_, 2)
            xt = xp_tiles[P_]
            for d in range(2):
                nt.matmul(pa[:, t_:t_ + 1], xt[:, 64 * d + 32 * s_:64 * d + 32 * s_ + 32],
                          u1_sb[:, d:d + 1], start=(d == 0), stop=False)
            for c in range(4):
                nt.matmul(pa[:, t_:t_ + 1],
                          hsT[:, t_ * 128 + 32 * c:t_ * 128 + 32 * (c + 1)],
                          u2_sb[:, c:c + 1], start=False, stop=(c == 3))

        z_tiles = {}

        def zslice(tiles, g, s):
            """(tile, col) for gate-chunk g, step-in-pair s."""
            zif, zg_, zo_ = tiles
            if g < 8:
                return zif, 256 * s + 32 * g
            if g < 12:
                return zo_, 128 * s + 32 * (g - 8)
            return zg_, 128 * s + 32 * (g - 12)

        def emit_pair_mm(P):
            """bias + xw matmuls for pair P into fresh per-gate-group PSUM tiles."""
            zif = ps_z.tile([128, 512], F32, tag="zif", name=f"zif{P}")
            zg_ = ps_z.tile([128, 256], F32, tag="zg", name=f"zg{P}")
            zo_ = ps_z.tile([128, 256], F32, tag="zo", name=f"zo{P}")
            tiles = (zif, zg_, zo_)
            xt = xp_tiles[P]
            nt.matmul(zif[:], biasT_sb, indif_sb, start=True, stop=False)
            nt.matmul(zg_[:], biasT_sb, indg_sb, start=True, stop=False)
            nt.matmul(zo_[:], biasT_sb, indo_sb, start=True, stop=False)
            for s in range(2):
                t_ = 2 * P + s
                for d in range(2):
                    xs = xt[:, 64 * d + 32 * s:64 * d + 32 * s + 32]
                    for g in range(16):
                        # no h-passes only for t=0: its regions stop at d==1
                        ztile, col = zslice(tiles, g, s)
                        nt.matmul(ztile[:, col:col + 32],
                                  wih_sb[:, (16 * d + g) * 128:(16 * d + g + 1) * 128],
                                  xs, start=False,
                                  stop=(t_ == 0 and d == 1))
            z_tiles[P] = tiles
            return tiles

        for Pp in range(4):
            dma_pair(Pp)
        # whh is first needed at step 1; pooling consts at the post-loop —
        # their DMAs go after the x prefetch so step 0 starts early
        whh_sb = consts.tile([128, 4 * 16 * 128], BF16, tag="whh")
        nc.sync.dma_start(whh_sb[:], d_whh[:])
        i128b_sb = consts.tile([128, 128], BF16, tag="i128b")
        nc.sync.dma_start(i128b_sb[:], d_i128b[:])
        i32s_sb = consts.tile([128, 32], F32, tag="i32s")
        nc.sync.dma_start(i32s_sb[:], d_i32s[:])
        maskneg_sb = consts.tile([BL, K * t_steps], F32, tag="maskneg")
        nc.sync.dma_start(maskneg_sb[:], d_maskneg[:])
        valid_sb = consts.tile([BL, K], F32, tag="valid")
        nc.sync.dma_start(valid_sb[:], d_valid[:])
        emit_pair_mm(0)
        emit_pair_mm(1)

        for t in range(t_steps):
            P, s = divmod(t, 2)
            zif, zg_, zo_ = z_tiles[P]
            # prefetch DMA + next pair's xw first: PE drains them during the
            # PREVIOUS step's gate chain, before stalling on h(t-1)
            if s == 0:
                if P + 4 < np_pairs:
                    dma_pair(P + 4)
                xp_tiles.pop(P - 2, None)
                z_tiles.pop(P - 1, None)
            # h-passes: contiguous on PE, gate-group order i,f -> g -> o;
            # per-gate-group PSUM tiles keep each activation's wait narrow
            hbase = (t - 1) * 128

            def hp(g, c):
                ztile, col = zslice(z_tiles[P], g, s)
                nt.matmul(ztile[:, col:col + 32],
                          whh_sb[:, (16 * c + g) * 128:(16 * c + g + 1) * 128],
                          hsT[:, hbase + 32 * c:hbase + 32 * (c + 1)],
                          start=False, stop=(c == 3))

            if t > 0:
                # i,f passes on h-chunks 0,1 first (ready before chunks 2,3)
                for g in range(8):
                    hp(g, 0)
                    hp(g, 1)
                for g in range(8):
                    hp(g, 2)
                    hp(g, 3)
                for g in [12, 13, 14, 15, 8, 9, 10, 11]:
                    for c in range(4):
                        hp(g, c)
            # gates
            sg = gate_pool.tile([128, 256], F32, tag="sg")
            ns.activation(sg[:], zif[:, 256 * s:256 * (s + 1)], AFT.Sigmoid)
            gg = gate_pool.tile([128, 128], F32, tag="gg")
            ns.activation(gg[:], zg_[:, 128 * s:128 * (s + 1)], AFT.Tanh)
            so = gate_pool.tile([128, 128], F32, tag="so")
            ns.activation(so[:], zo_[:, 128 * s:128 * (s + 1)], AFT.Sigmoid)
            if t == 0:
                nv.tensor_tensor(cT[:], sg[:, 0:128], gg[:], op=ALU.mult)
            else:
                tfc = tmp_pool.tile([128, 128], F32, tag="tfc")
                nv.tensor_tensor(tfc[:], sg[:, 128:256], cT[:], op=ALU.mult)
                tig = tmp_pool.tile([128, 128], F32, tag="tig")
                for hh in range(2):
                    sl = slice(64 * hh, 64 * (hh + 1))
                    nv.tensor_tensor(tig[:, sl], sg[:, 0:128][:, sl], gg[:, sl],
                                     op=ALU.mult)
                    nv.tensor_tensor(cT[:, sl], tfc[:, sl], tig[:, sl], op=ALU.add)
                # a_pre for step t-1 (h/x ready; fills PE idle in the chain)
                emit_a(t - 1)
            # tail in halves: h-chunks 0,1 land early so next step's c=0,1
            # matmul passes can start while the second half finishes
            tcs = tmp_pool.tile([128, 128], F32, tag="tcs")
            for hh in range(2):
                sl = slice(64 * hh, 64 * (hh + 1))
                ns.activation(tcs[:, sl], cT[:, sl], AFT.Tanh)
                nv.tensor_tensor(hsT[:, t * 128 + 64 * hh:t * 128 + 64 * (hh + 1)],
                                 so[:, sl], tcs[:, sl], op=ALU.mult)
            # pair emission on ODD iters: the 67 bias+xw matmuls drain during
            # this step's gate chain instead of delaying the next (even)
            # step's h-passes on the in-order PE
            if s == 1:
                Pn = (t + 1) // 2 + 1
                if Pn < np_pairs:
                    emit_pair_mm(Pn)

        # final a_pre for last step
        emit_a(t_steps - 1)
        # a = sigmoid(a1 + a2), to SBUF before psum pools close
        ns.activation(abp[:], pa[:], AFT.Sigmoid)
        loop_ctx.close()

        # ---- post-loop: windowed softmax + pooling ----
        post = ctx.enter_context(tc.tile_pool(name="post", bufs=1))
        ps_t = ctx.enter_context(tc.tile_pool(name="ps_t", bufs=2, space="PSUM"))
        ps_pool = ctx.enter_context(tc.tile_pool(name="ps_pool", bufs=3, space="PSUM"))
        hsb_pool = ctx.enter_context(tc.tile_pool(name="hsb", bufs=4))
        stg_pool = ctx.enter_context(tc.tile_pool(name="stg", bufs=6))

        # hoist the first pooling transposes ahead of the softmax: they only
        # need hsT, and the in-order PE would otherwise park them behind the
        # wk transposes that wait for the whole softmax chain
        hsT_r = hsT[:].rearrange("p (t c b) -> p t c b", c=4, b=BL)
        pts = {}
        for b in range(3):
            pt = ps_t.tile([128, 512], BF16, tag="pt", bufs=4, name=f"pt{b}")
            for c in range(4):
                nt.transpose(pt[0:t_steps, 128 * c:128 * (c + 1)],
                             hsT_r[:, :, c, b], i128b_sb[:])
            pts[b] = pt

        # softmax per slot k -> wT [t, 4b+k] (bf16 for the pooling matmul)
        # batched over all 4 slots; logits are sigmoid outputs in (0,1) so
        # exp never overflows (no max-subtract) and masked -1e9 -> exp = 0
        wT = post.tile([t_steps, K * BL], BF16, tag="wT")
        scb = post.tile([BL, K * t_steps], F32, tag="scb")
        a_b = abp[:].rearrange("b (k t) -> b k t", k=1).broadcast_to([BL, K, t_steps])
        nv.tensor_tensor(scb[:].rearrange("b (k t) -> b k t", k=K), a_b,
                         maskneg_sb[:].rearrange("b (k t) -> b k t", k=K),
                         op=ALU.add)
        ekb = post.tile([BL, K * t_steps], F32, tag="ekb")
        ns.activation(ekb[:], scb[:], AFT.Exp)
        sk4 = post.tile([BL, K], F32, tag="sk4")
        nv.tensor_reduce(sk4[:], ekb[:].rearrange("b (k t) -> b k t", k=K),
                         axis=mybir.AxisListType.X, op=ALU.add)
        # all-masked (invalid) slots sum to 0 -> eps keeps 1/sum finite; the
        # valid-mask multiply below zeroes them exactly
        nv.tensor_scalar(out=sk4[:], in0=sk4[:], scalar1=1e-30, scalar2=None, op0=ALU.add)
        rk4 = post.tile([BL, K], F32, tag="rk4")
        nv.reciprocal(rk4[:], sk4[:])
        nv.tensor_tensor(rk4[:], rk4[:], valid_sb[:], op=ALU.mult)
        wkb = post.tile([BL, K * t_steps], F32, tag="wkb")
        r_b = rk4[:].rearrange("b (k o) -> b k o", o=1).broadcast_to([BL, K, t_steps])
        nv.tensor_tensor(wkb[:].rearrange("b (k t) -> b k t", k=K),
                         ekb[:].rearrange("b (k t) -> b k t", k=K), r_b,
                         op=ALU.mult)
        pwT = ps_t.tile([128, K * BL], F32, tag="pwT", bufs=1)
        for k in range(K):
            nt.transpose(pwT[0:t_steps, 32 * k:32 * (k + 1)],
                         wkb[:, t_steps * k:t_steps * (k + 1)], i32s_sb[0:32, :])
        nv.tensor_copy(wT[:].rearrange("t (b k) -> t b k", k=K),
                       pwT[0:t_steps, :].rearrange("t (k b) -> t b k", b=BL))
        # per-b zero-padded [T,16] weight slices for the grouped pool matmuls
        wTm_all = post.tile([t_steps, BL * 4 * K], BF16, tag="wTm_all")
        nv.memset(wTm_all[:], 0.0)

        # pooling: per b, rebuild hs_b [t, h] via 4 PE transposes into ONE
        # psum tile + one [128,512] copy (Act/DVE alternated). Pool matmuls
        # run in groups of 4 b's accumulating one [16,512] PSUM tile: each
        # matmul's weights are a zero-padded [T,16] slice (only cols 4j..4j+4
        # live), so cross-b terms vanish; one [16,512] copy + DMA per group.
        def emit_transposes(bb):
            ptn = ps_t.tile([128, 512], BF16, tag="pt", bufs=4, name=f"pt{bb}")
            for c in range(4):
                nt.transpose(ptn[0:t_steps, 128 * c:128 * (c + 1)],
                             hsT_r[:, :, c, bb], i128b_sb[:])
            pts[bb] = ptn

        for b in range(BL):
            # keep a 3-deep transpose lead so PE never waits behind pool mms
            if b + 3 < BL and (b + 3) not in pts:
                emit_transposes(b + 3)
            pt = pts.pop(b)
            hsb = hsb_pool.tile([t_steps, H], BF16, tag="hsb")
            if b % 4 == 0:
                ns.copy(hsb[:], pt[0:t_steps, :])
            else:
                nv.tensor_copy(hsb[:], pt[0:t_steps, :])
            j = b % 4
            wTm = wTm_all[:, 16 * b:16 * (b + 1)]
            nv.tensor_copy(wTm[:, 4 * j:4 * (j + 1)],
                           wT[0:t_steps, 4 * b:4 * (b + 1)])
            if j == 0:
                pp4 = ps_pool.tile([4 * K, H], F32, tag="pp4", name=f"pp4_{b}")
                pp4_hold = pp4
            else:
                pp4 = pp4_hold
            nt.matmul(pp4[:], wTm, hsb[:], start=(j == 0), stop=(j == 3))
            if j == 3:
                so4 = stg_pool.tile([4 * K, H], F32, tag="so4", name=f"so4_{b}")
                ns.copy(so4[:], pp4[:])
                nc.sync.dma_start(d_out[K * (b - 3):K * (b + 1), :], so4[:])

    nc.compile()
    return nc


def _host_prep(x, W_ih, W_hh, b_ih, b_hh, A1, A2, v1, lengths, label_len):
    assert int(label_len) == K
    BF16n = ml_dtypes.bfloat16
    perm = np.concatenate([np.arange(0, 512), np.arange(512, 1024),
                           np.arange(1536, 2048), np.arange(1024, 1536)])
    wih_f = np.ascontiguousarray(W_ih[perm].T, dtype=np.float32)   # [256, 2048]
    whh_f = np.ascontiguousarray(W_hh[perm].T, dtype=np.float32)   # [512, 2048]
    # blocks: wih[d-chunk, g-chunk] -> [128, (16d+g)*128 + j]
    wih = wih_f.reshape(2, 128, 16, 128).transpose(1, 0, 2, 3).reshape(128, -1)
    whh = whh_f.reshape(4, 128, 16, 128).transpose(1, 0, 2, 3).reshape(128, -1)
    bias = ((b_ih + b_hh)[perm]).astype(np.float32)
    biasT = bias.reshape(16, 128)                                  # [k, p]
    indif = np.zeros((16, 2, 8, 32), dtype=np.float32)
    indg = np.zeros((16, 2, 4, 32), dtype=np.float32)
    indo = np.zeros((16, 2, 4, 32), dtype=np.float32)
    for kk in range(8):
        indif[kk, :, kk, :] = 1.0
    for kk in range(4):
        indg[12 + kk, :, kk, :] = 1.0
        indo[8 + kk, :, kk, :] = 1.0
    indif = indif.reshape(16, 512)
    indg = indg.reshape(16, 256)
    indo = indo.reshape(16, 256)
    u1 = (v1 @ A1)[0].astype(np.float32)                           # [256]
    u2 = (v1 @ A2)[0].astype(np.float32)                           # [512]
    u1m = u1.reshape(2, 128).T.copy()                              # [128, 2]
    u2m = u2.reshape(4, 128).T.copy()                              # [128, 4]
    i128b = np.eye(128, dtype=np.float32)
    i32s = np.zeros((128, 32), dtype=np.float32)
    i32s[np.arange(128), np.arange(128) % 32] = 1.0

    pack16 = np.concatenate([biasT, indif, indg, indo], axis=1)   # [16, 1152]
    u12 = np.concatenate([u1m, u2m], axis=1)                      # [128, 6]
    shared = dict(wih=wih.astype(BF16n), whh=whh.astype(BF16n),
                  pack16=pack16.astype(BF16n), u12=u12.astype(BF16n),
                  i128b=i128b.astype(BF16n), i32s=i32s)

    in_maps = []
    for cidx in range(NC):
        sl = slice(cidx * BL, (cidx + 1) * BL)
        xc = x[:, sl, :]                                           # [T, 32, D]
        xT = np.ascontiguousarray(xc.reshape(T * BL, D).T, dtype=np.float32)
        ln = lengths[sl].astype(np.int64)
        t_start = np.maximum(ln - K, 0)
        t_k = t_start[:, None] + np.arange(K)[None, :]             # [32, 4]
        valid = (t_k <= (ln[:, None] - 1))                         # [32, 4]
        tt = np.arange(T)
        mask = (tt[None, None, :] <= t_k[:, :, None]) & valid[:, :, None]
        maskneg = np.where(mask, 0.0, NEG_INF).astype(np.float32)
        maskneg = np.ascontiguousarray(maskneg.reshape(BL, K * T))
        in_maps.append(dict(shared, xT=xT.astype(BF16n), maskneg=maskneg,
                            valid=valid.astype(np.float32)))
    return in_maps


def kernel(**inputs) -> np.ndarray:
    inputs = {k: np.asarray(v) if not np.isscalar(v) else v for k, v in inputs.items()}
    in_maps = _host_prep(**inputs)
    if "nc" not in _cached:
        _cached["nc"] = _build_program()
    nc = _cached["nc"]
    res = run_bass_kernel_spmd(nc, in_maps, core_ids=list(range(NC)))
    outs = []
    for cidx in range(NC):
        o = res.results[cidx]["out"]                               # [128, 512]
        outs.append(o.reshape(BL, K, H))
    return np.concatenate(outs, axis=0).astype(np.float32)         # [256, 4, 512]
